# revision 5
# baseline (speedup 1.0000x reference)
"""Trainium2 Bass kernel for an 8-head post-norm transformer block.

Contract: kernel(**inputs) takes the FULL inputs from setup_inputs()
(x [64,256,512], per-head QKV weights, Wo, FFN weights, LN params) and
returns the FULL [64,256,512] output, computed on 8 NeuronCores.

Sharding: pure data-parallel over the batch dim - 8 batches per core,
no collectives. Each core runs an identical program on its own slice.

v3 structure (per core, 2048 tokens, all matmuls bf16):
  - xT fed pre-transposed from host as 4 per-batch-pair tiles (no
    whole-tile DMA dependency at startup), w1/w2 prefetched at t=0
  - causal mask applied ON the PE: one extra 128-col matmul accumulates
    -1000 * 1[q < p] (strictL @ -1000*I) onto each triangular scores
    block, so exp() output needs no elementwise masking at all
  - LayerNorm rstd = exp(-0.5*ln(var+eps)) on the scalar engine: Exp and
    Ln live in the same activation table set, so the Exp<->Sqrt table
    thrash (1.28us per swap) is eliminated
  - single software-pipelined loop over 4 batch-pairs (bp = 2 batches,
    512 tokens): QKV+attention(bp) interleaved -> FFN(bp-1) -> rest of
    attention -> proj+LN1(bp); FFN(3) drains after the loop
  - attention: fully-masked kc1/q<128 block skipped; both heads of a
    pair packed into one [128,*] AV+denominator PSUM via tile_position
    column offset; softmax normalize is one full-width vector mul
  PSUM: scores 2 + AV/denom 2 + (QKV/proj) 2 + (FFN1/FFN2) 2 = 8 banks.
"""
import sys

if '/opt/trn_rl_repo' not in sys.path:
    sys.path.insert(0, '/opt/trn_rl_repo')

import numpy as np

D, DFF, H, E, T = 512, 2048, 8, 64, 256
NCORES = 8
BPC = 8            # batches per core
TOK = BPC * T      # 2048 tokens per core
NT = TOK // 128    # 16 token tiles
DC = D // 128      # 4 feature chunks
FC = DFF // 128    # 16 dff chunks
NBP = BPC // 2     # 4 batch-pairs (512 tokens each)
NEG = -1000.0      # causal-mask additive constant (exp(0.125*-990) -> 0)

_cached = None


def _build_program():
    import concourse.mybir as mybir
    import concourse.tile as tile
    from concourse import bacc

    f32 = mybir.dt.float32
    bf16 = mybir.dt.bfloat16
    AF = mybir.ActivationFunctionType
    ALU = mybir.AluOpType

    nc = bacc.Bacc("TRN2", target_bir_lowering=False, debug=False,
                   num_devices=NCORES)

    def din(name, shape, dt=None):
        return nc.dram_tensor(name, shape, dt or f32, kind="ExternalInput").ap()

    x_d = din("x", [NT, 128, D])
    xT_d = din("xT", [NBP, 128, DC, 512], bf16)
    wq_d = din("wq", [128, DC, D], bf16)      # [d-part, d-chunk, hE]
    wk_d = din("wk", [128, DC, D], bf16)
    wv_d = din("wv", [128, DC, D], bf16)
    wo_d = din("wo", [128, DC, D], bf16)      # [hE-part, hE-chunk, d]
    w1_d = din("w1", [128, DC, DFF], bf16)    # gamma1-folded on host
    w2_d = din("w2", [128, FC, D], bf16)
    b1_d = din("b1t", [128, FC])              # b1 + W1.T@ln1_b, per dff-part
    g1_d = din("g1b", [128, D])
    be1_d = din("be1b", [128, D])             # ln1_b + b2 (host-folded)
    g2_d = din("g2b", [128, D])
    be2_d = din("be2b", [128, D])
    sL_d = din("strictL", [128, 128], bf16)   # [k,p] = 1 if p > k
    nI_d = din("negI", [128, 128], bf16)      # NEG * identity
    ones_d = din("ones64", [128, 64], bf16)
    eps_d = din("eps", [128, 1])
    y_d = nc.dram_tensor("y", [NT, 128, D], f32, kind="ExternalOutput").ap()

    def mm(out, lhsT, rhs, start, stop):
        nc.tensor.matmul(out, lhsT, rhs, start=start, stop=stop,
                         skip_group_check=True)

    with tile.TileContext(nc) as tc:
        _cms = []

        def _open(**kw):
            cm = tc.tile_pool(**kw)
            pool = cm.__enter__()
            _cms.append(cm)
            return pool

        # ---------------- persistent SBUF + weight prefetch --------------
        consts = _open(name="consts", bufs=1)
        ones64 = consts.tile([128, 64], bf16, tag="ones64", name="ones64")
        strictL = consts.tile([128, 128], bf16, tag="sL", name="sL")
        negI = consts.tile([128, 128], bf16, tag="nI", name="nI")
        b1t = consts.tile([128, FC], f32, tag="b1t", name="b1t")
        g1b = consts.tile([128, D], f32, tag="g1b", name="g1b")
        be1b = consts.tile([128, D], f32, tag="be1b", name="be1b")
        g2b = consts.tile([128, D], f32, tag="g2b", name="g2b")
        be2b = consts.tile([128, D], f32, tag="be2b", name="be2b")
        epsb = consts.tile([128, 1], f32, tag="eps", name="eps")

        pw = _open(name="pw", bufs=1)
        xTt = [pw.tile([128, DC, 512], bf16, tag=f"xT{c}", name=f"xT{c}")
               for c in range(NBP)]
        wq_sb = pw.tile([128, DC, D], bf16, tag="wq", name="wq")
        wk_sb = pw.tile([128, DC, D], bf16, tag="wk", name="wk")
        wv_sb = pw.tile([128, DC, D], bf16, tag="wv", name="wv")
        wo_sb = pw.tile([128, DC, D], bf16, tag="wo", name="wo")
        w1_sb = pw.tile([128, DC, DFF], bf16, tag="w1", name="w1")
        w2_sb = pw.tile([128, FC, D], bf16, tag="w2", name="w2")
        ln1_sb = pw.tile([128, NT, D], bf16, tag="ln1", name="ln1")

        # latency-critical first on the sync queue
        nc.sync.dma_start(wq_sb[:], wq_d[:])
        nc.sync.dma_start(xTt[0][:], xT_d[0])
        nc.sync.dma_start(wk_sb[:], wk_d[:])
        nc.sync.dma_start(wv_sb[:], wv_d[:])
        for c in range(1, NBP):
            nc.sync.dma_start(xTt[c][:], xT_d[c])
        nc.sync.dma_start(wo_sb[:], wo_d[:])
        # big FFN weights + small consts in background on the scalar queue
        nc.scalar.dma_start(w1_sb[:], w1_d[:])
        nc.scalar.dma_start(w2_sb[:], w2_d[:])
        for t_, d_ in ((strictL, sL_d), (negI, nI_d), (ones64, ones_d),
                       (b1t, b1_d), (g1b, g1_d), (be1b, be1_d),
                       (g2b, g2_d), (be2b, be2_d), (epsb, eps_d)):
            nc.scalar.dma_start(t_[:], d_[:])

        # ------------------------------ pools -----------------------------
        pqk = _open(name="pqk", bufs=2)
        pvb = _open(name="pvb", bufs=2)
        pPT = _open(name="pPT", bufs=4)
        phT = _open(name="phT", bufs=2)
        pln1T = _open(name="pln1T", bufs=2)
        ph1 = _open(name="ph1", bufs=1)
        prec = _open(name="prec", bufs=3)
        pxs = _open(name="pxs", bufs=5)
        pres = _open(name="pres", bufs=4)
        lntmp = _open(name="lntmp", bufs=3)
        pyout = _open(name="pyout", bufs=3)
        lnstat = _open(name="lnstat", bufs=6)
        pscr = _open(name="pscr", bufs=2, space="PSUM")
        pavd = _open(name="pavd", bufs=2, space="PSUM")
        pbig = _open(name="pbig", bufs=2, space="PSUM")
        pff1 = _open(name="pff1", bufs=2, space="PSUM")

        def ln_core(in_ap, out_dt):
            """Normalize (x-mean)*rstd; rstd via exp(-0.5*ln(var+eps))."""
            st = lnstat.tile([128, 6], f32, tag="st", name="st")
            nc.vector.bn_stats(st[:], in_ap)
            mv = lnstat.tile([128, 2], f32, tag="mv", name="mv")
            nc.vector.bn_aggr(mv[:], st[:])
            lnv = lnstat.tile([128, 1], f32, tag="lnv", name="lnv")
            nc.scalar.activation(lnv[:], mv[:, 1:2], AF.Ln, bias=epsb[:, 0:1])
            rstd = lnstat.tile([128, 1], f32, tag="rstd", name="rstd")
            nc.scalar.activation(rstd[:], lnv[:], AF.Exp, scale=-0.5)
            nmr = lnstat.tile([128, 1], f32, tag="nmr", name="nmr")
            nc.vector.tensor_scalar_mul(nmr[:], mv[:, 0:1], -1.0)
            tmp = lntmp.tile([128, D], out_dt, tag="lnt", name="lnt")
            nc.vector.tensor_scalar(tmp[:], in_ap, nmr[:, 0:1], rstd[:, 0:1],
                                    ALU.add, ALU.mult)
            return tmp

        ln1T_tiles = [None] * NBP

        def qkv_qk(bp, m, qT2, kT2):
            for w_sb, dst in ((wq_sb, qT2), (wk_sb, kT2)):
                ps = pbig.tile([128, 512], f32, tag="pbig", name="pbig")
                for k in range(DC):
                    mm(ps[:], w_sb[:, k, m * 128:(m + 1) * 128],
                       xTt[bp][:, k, :], start=k == 0, stop=k == DC - 1)
                nc.scalar.copy(dst[:, m, :], ps[:])

        def qkv_v(bp, j, vb4):
            ps = pbig.tile([128, 512], f32, tag="pbig", name="pbig")
            for k in range(DC):
                mm(ps[:], xTt[bp][:, k, j * 128:(j + 1) * 128],
                   wv_sb[:, k, :], start=k == 0, stop=k == DC - 1)
            nc.vector.tensor_scalar_mul(vb4[:, j, :], ps[:], 1.0)

        def emit_ffn(bq):
            """FFN + LN2 + store for batch-pair bq (ln1T/ln1_sb ready)."""
            lt = ln1T_tiles[bq]
            h1 = ph1.tile([128, FC, 512], bf16, tag="h1", name="h1")
            for m in range(FC):
                ps = pff1.tile([128, 512], f32, tag="pff1", name="pff1")
                for k in range(DC):
                    mm(ps[:], w1_sb[:, k, m * 128:(m + 1) * 128],
                       lt[:, k, :], start=k == 0, stop=k == DC - 1)
                if m % 2 == 0:
                    nc.scalar.activation(h1[:, m, :], ps[:], AF.Relu,
                                         bias=b1t[:, m:m + 1])
                else:
                    nc.vector.tensor_scalar(h1[:, m, :], ps[:],
                                            b1t[:, m:m + 1], 0.0,
                                            ALU.add, ALU.max)
            for j in range(4):
                t = 4 * bq + j
                ps2 = pff1.tile([128, 512], f32, tag="pff1", name="pff1")
                for k in range(FC):
                    mm(ps2[:], h1[:, k, j * 128:(j + 1) * 128], w2_sb[:, k, :],
                       start=k == 0, stop=k == FC - 1)
                res2 = pres.tile([128, D], f32, tag="res", name="res")
                nc.vector.tensor_add(res2[:], ps2[:], ln1_sb[:, t, :])
                tmp2 = ln_core(res2[:], f32)
                yt = pyout.tile([128, D], f32, tag="yt", name="yt")
                nc.gpsimd.tensor_mul(yt[:], tmp2[:], g2b[:])
                nc.gpsimd.tensor_add(yt[:], yt[:], be2b[:])
                nc.sync.dma_start(y_d[t], yt[:])

        def emit_scores(u, qT2, kT2):
            """scores (+PE causal mask) + exp for unit u; returns P tile."""
            bi, pair = u // 4, u % 4
            q0 = bi * 256
            ptr = pPT.tile([128, 2, 2, 256], bf16, tag="ptr", name="ptr")
            nc.gpsimd.memset(ptr[:, :, 1, 0:128], 0.0)
            for hh in range(2):
                r0 = hh * 64
                sc = pscr.tile([128, 512], f32, tag="sc", name="sc")
                # kc0: keys 0:128, all 256 queries; mask q<128 triangle
                mm(sc[:, 0:256], kT2[r0:r0 + 64, pair, q0:q0 + 128],
                   qT2[r0:r0 + 64, pair, q0:q0 + 256],
                   start=True, stop=False)
                mm(sc[:, 0:128], strictL[:], negI[:], start=False, stop=True)
                # kc1: keys 128:256, queries 128:256 only (causal trim)
                mm(sc[:, 384:512], kT2[r0:r0 + 64, pair, q0 + 128:q0 + 256],
                   qT2[r0:r0 + 64, pair, q0 + 128:q0 + 256],
                   start=True, stop=False)
                mm(sc[:, 384:512], strictL[:], negI[:], start=False, stop=True)
                # exp(s/8) straight to the bf16 P tile (masked lanes -> 0)
                nc.scalar.activation(ptr[:, hh, 0, :], sc[:, 0:256],
                                     AF.Exp, scale=0.125)
                nc.scalar.activation(ptr[:, hh, 1, 128:256], sc[:, 384:512],
                                     AF.Exp, scale=0.125)
            return ptr

        def emit_av(u, ptr, vb4, headsT):
            """AV + denominator (hh-packed) + normalize for unit u."""
            bi, pair = u // 4, u % 4
            avd = pavd.tile([128, 512], f32, tag="avd", name="avd")
            for hh in range(2):
                h = 2 * pair + hh
                c0 = hh * 64
                he = slice(h * E, (h + 1) * E)
                o = avd[c0:c0 + 64, :]
                mm(o[:, 0:128], vb4[:, 2 * bi, he], ptr[:, hh, 0, 0:128],
                   start=True, stop=True)
                mm(o[:, 128:256], vb4[:, 2 * bi, he], ptr[:, hh, 0, 128:256],
                   start=True, stop=False)
                mm(o[:, 128:256], vb4[:, 2 * bi + 1, he],
                   ptr[:, hh, 1, 128:256], start=False, stop=True)
                mm(o[:, 256:384], ones64[:], ptr[:, hh, 0, 0:128],
                   start=True, stop=True)
                mm(o[:, 384:512], ones64[:], ptr[:, hh, 0, 128:256],
                   start=True, stop=False)
                mm(o[:, 384:512], ones64[:], ptr[:, hh, 1, 128:256],
                   start=False, stop=True)
            rec = prec.tile([128, 256], f32, tag="rec", name="rec")
            nc.vector.reciprocal_approx_fast(rec[:], avd[:, 256:512])
            nc.vector.tensor_mul(headsT[:, pair, bi * 256:(bi + 1) * 256],
                                 avd[:, 0:256], rec[:])

        def emit_proj(bp, j, headsT, ln1T, xin):
            t = 4 * bp + j
            ps = pbig.tile([128, 512], f32, tag="pbig", name="pbig")
            for k in range(DC):
                mm(ps[:], headsT[:, k, j * 128:(j + 1) * 128],
                   wo_sb[:, k, :], start=k == 0, stop=k == DC - 1)
            res = pres.tile([128, D], f32, tag="res", name="res")
            nc.vector.tensor_add(res[:], ps[:], xin[:])
            tmp = ln_core(res[:], bf16)
            nc.sync.dma_start_transpose(
                ln1T[:, :, j * 128:(j + 1) * 128], tmp[:])
            nc.gpsimd.tensor_mul(ln1_sb[:, t, :], tmp[:], g1b[:])
            nc.gpsimd.tensor_add(ln1_sb[:, t, :], ln1_sb[:, t, :], be1b[:])

        # ------------------------------ main loop -------------------------
        for bp in range(NBP):
            xins = []
            for j in range(4):
                xin = pxs.tile([128, D], f32, tag="xs", name="xs")
                nc.sync.dma_start(xin[:], x_d[4 * bp + j])
                xins.append(xin)
            qT2 = pqk.tile([128, DC, 512], bf16, tag="q", name="qT2")
            kT2 = pqk.tile([128, DC, 512], bf16, tag="k", name="kT2")
            vb4 = pvb.tile([128, 4, D], bf16, tag="v", name="vb4")
            headsT = phT.tile([128, DC, 512], bf16, tag="hT", name="hT")
            ln1T = pln1T.tile([128, DC, 512], bf16, tag="l1T", name="l1T")
            ln1T_tiles[bp] = ln1T

            qkv_qk(bp, 0, qT2, kT2)
            qkv_qk(bp, 1, qT2, kT2)
            qkv_v(bp, 0, vb4)
            qkv_v(bp, 1, vb4)
            ptrs = [None] * 8
            ptrs[0] = emit_scores(0, qT2, kT2)
            qkv_qk(bp, 2, qT2, kT2)
            ptrs[1] = emit_scores(1, qT2, kT2)
            emit_av(0, ptrs[0], vb4, headsT)
            qkv_qk(bp, 3, qT2, kT2)
            ptrs[2] = emit_scores(2, qT2, kT2)
            emit_av(1, ptrs[1], vb4, headsT)
            qkv_v(bp, 2, vb4)
            qkv_v(bp, 3, vb4)
            ptrs[3] = emit_scores(3, qT2, kT2)
            emit_av(2, ptrs[2], vb4, headsT)
            ptrs[4] = emit_scores(4, qT2, kT2)
            if bp > 0:
                emit_ffn(bp - 1)
            emit_av(3, ptrs[3], vb4, headsT)
            ptrs[5] = emit_scores(5, qT2, kT2)
            emit_proj(bp, 0, headsT, ln1T, xins[0])
            emit_av(4, ptrs[4], vb4, headsT)
            ptrs[6] = emit_scores(6, qT2, kT2)
            emit_proj(bp, 1, headsT, ln1T, xins[1])
            emit_av(5, ptrs[5], vb4, headsT)
            ptrs[7] = emit_scores(7, qT2, kT2)
            emit_av(6, ptrs[6], vb4, headsT)
            emit_av(7, ptrs[7], vb4, headsT)
            emit_proj(bp, 2, headsT, ln1T, xins[2])
            emit_proj(bp, 3, headsT, ln1T, xins[3])
        emit_ffn(NBP - 1)

        for cm in reversed(_cms):
            cm.__exit__(None, None, None)

    nc.finalize()
    return nc


def _host_prep(inputs):
    """Build the per-core in_maps from full inputs."""
    import ml_dtypes
    bf = ml_dtypes.bfloat16
    x = np.ascontiguousarray(np.asarray(inputs["x"], np.float32))
    Wq = np.asarray(inputs["Wq"], np.float32)
    Wk = np.asarray(inputs["Wk"], np.float32)
    Wv = np.asarray(inputs["Wv"], np.float32)
    Wo = np.asarray(inputs["Wo"], np.float32)
    W1 = np.asarray(inputs["W1"], np.float32)
    b1 = np.asarray(inputs["b1"], np.float32)
    W2 = np.asarray(inputs["W2"], np.float32)
    b2 = np.asarray(inputs["b2"], np.float32)
    g1 = np.asarray(inputs["ln1_g"], np.float32)
    be1 = np.asarray(inputs["ln1_b"], np.float32)
    g2 = np.asarray(inputs["ln2_g"], np.float32)
    be2 = np.asarray(inputs["ln2_b"], np.float32)

    def chunk_k(w, dt):   # [K, M] -> [128, K//128, M]
        K, M = w.shape
        return np.ascontiguousarray(
            w.reshape(K // 128, 128, M).transpose(1, 0, 2).astype(dt))

    W1g = g1[:, None] * W1                 # fold ln1 gamma into W1
    b1_eff = b1 + be1 @ W1                 # fold ln1 beta into FFN1 bias

    common = {
        "wq": chunk_k(Wq.transpose(1, 0, 2).reshape(D, H * E), bf),
        "wk": chunk_k(Wk.transpose(1, 0, 2).reshape(D, H * E), bf),
        "wv": chunk_k(Wv.transpose(1, 0, 2).reshape(D, H * E), bf),
        "wo": chunk_k(Wo, bf),
        "w1": chunk_k(W1g, bf),
        "w2": chunk_k(W2, bf),
        "b1t": np.ascontiguousarray(b1_eff.reshape(FC, 128).T
                                    .astype(np.float32)),
        "g1b": np.ascontiguousarray(np.tile(g1, (128, 1))),
        "be1b": np.ascontiguousarray(np.tile(be1 + b2, (128, 1))),
        "g2b": np.ascontiguousarray(np.tile(g2, (128, 1))),
        "be2b": np.ascontiguousarray(np.tile(be2, (128, 1))),
        "strictL": (np.arange(128)[None, :] > np.arange(128)[:, None])
            .astype(bf),
        "negI": (NEG * np.eye(128)).astype(bf),
        "ones64": np.ones((128, 64), bf),
        "eps": np.full((128, 1), 1e-5, np.float32),
    }
    in_maps = []
    for core in range(NCORES):
        xc = x[core * BPC:(core + 1) * BPC].reshape(NT, 128, D)
        xTc = np.ascontiguousarray(
            xc.reshape(TOK, D).T.reshape(DC, 128, NBP, 512)
            .transpose(2, 1, 0, 3).astype(bf))
        in_maps.append({"x": np.ascontiguousarray(xc), "xT": xTc, **common})
    return in_maps


def _get_program():
    global _cached
    if _cached is None:
        _cached = _build_program()
    return _cached


def _run(inputs, trace=False):
    from concourse.bass_utils import run_bass_kernel_spmd
    nc = _get_program()
    in_maps = _host_prep(inputs)
    res = run_bass_kernel_spmd(nc, in_maps, list(range(NCORES)), trace=trace)
    outs = [res.results[i]["y"].reshape(BPC, T, D) for i in range(NCORES)]
    return np.concatenate(outs, 0).astype(np.float32), res


def kernel(**inputs):
    out, _ = _run(inputs, trace=False)
    return out


# revision 10
# speedup vs baseline: 1.1276x; 1.1276x over previous
"""Trainium2 Bass kernel for an 8-head post-norm transformer block.

Contract: kernel(**inputs) takes the FULL inputs from setup_inputs()
(x [64,256,512], per-head QKV weights, Wo, FFN weights, LN params) and
returns the FULL [64,256,512] output, computed on 8 NeuronCores.

Sharding: pure data-parallel over the batch dim - 8 batches per core,
no collectives. Each core runs an identical program on its own slice.

v3 structure (per core, 2048 tokens, all matmuls bf16):
  - xT fed pre-transposed from host as 4 per-batch-pair tiles (no
    whole-tile DMA dependency at startup), w1/w2 prefetched at t=0
  - causal mask applied ON the PE: one extra 128-col matmul accumulates
    -1000 * 1[q < p] (strictL @ -1000*I) onto each triangular scores
    block, so exp() output needs no elementwise masking at all
  - LayerNorm rstd = exp(-0.5*ln(var+eps)) on the scalar engine: Exp and
    Ln live in the same activation table set, so the Exp<->Sqrt table
    thrash (1.28us per swap) is eliminated
  - single software-pipelined loop over 4 batch-pairs (bp = 2 batches,
    512 tokens): QKV+attention(bp) interleaved -> FFN(bp-1) -> rest of
    attention -> proj+LN1(bp); FFN(3) drains after the loop
  - attention: fully-masked kc1/q<128 block skipped; both heads of a
    pair packed into one [128,*] AV+denominator PSUM via tile_position
    column offset; softmax normalize is one full-width vector mul
  PSUM: scores 2 + AV/denom 2 + (QKV/proj) 2 + (FFN1/FFN2) 2 = 8 banks.
"""
import sys

if '/opt/trn_rl_repo' not in sys.path:
    sys.path.insert(0, '/opt/trn_rl_repo')

import numpy as np

D, DFF, H, E, T = 512, 2048, 8, 64, 256
NCORES = 8
BPC = 8            # batches per core
TOK = BPC * T      # 2048 tokens per core
NT = TOK // 128    # 16 token tiles
DC = D // 128      # 4 feature chunks
FC = DFF // 128    # 16 dff chunks
NBP = BPC // 2     # 4 batch-pairs (512 tokens each)
NEG = -1000.0      # causal-mask additive constant (exp(0.125*-990) -> 0)

_cached = None


def _build_program():
    import concourse.mybir as mybir
    import concourse.tile as tile
    from concourse import bacc

    f32 = mybir.dt.float32
    bf16 = mybir.dt.bfloat16
    AF = mybir.ActivationFunctionType
    ALU = mybir.AluOpType

    nc = bacc.Bacc("TRN2", target_bir_lowering=False, debug=False,
                   num_devices=NCORES)

    def din(name, shape, dt=None):
        return nc.dram_tensor(name, shape, dt or f32, kind="ExternalInput").ap()

    x_d = din("x", [NT, 128, D])
    xT_d = din("xT", [NBP, 128, DC, 512], bf16)
    wq_d = din("wq", [128, DC, D], bf16)      # [d-part, d-chunk, hE]
    wk_d = din("wk", [128, DC, D], bf16)
    wv_d = din("wv", [128, DC, D], bf16)
    wo_d = din("wo", [128, DC, D], bf16)      # [hE-part, hE-chunk, d]
    w1_d = din("w1", [128, DC, DFF], bf16)    # gamma1-folded on host
    w2_d = din("w2", [128, FC, D], bf16)
    b1_d = din("b1t", [128, FC])              # b1 + W1.T@ln1_b, per dff-part
    g1_d = din("g1b", [128, D])
    be1_d = din("be1b", [128, D])             # ln1_b + b2 (host-folded)
    g2_d = din("g2b", [128, D])
    be2_d = din("be2b", [128, D])
    sL_d = din("strictL", [128, 128], bf16)   # [k,p] = 1 if p > k
    nI_d = din("negI", [128, 128], bf16)      # NEG * identity
    ones_d = din("ones64", [128, 64], bf16)
    eps_d = din("eps", [128, 1])
    y_d = nc.dram_tensor("y", [NT, 128, D], f32, kind="ExternalOutput").ap()

    def mm(out, lhsT, rhs, start, stop):
        nc.tensor.matmul(out, lhsT, rhs, start=start, stop=stop,
                         skip_group_check=True)

    with tile.TileContext(nc) as tc:
        _cms = []

        def _open(**kw):
            cm = tc.tile_pool(**kw)
            pool = cm.__enter__()
            _cms.append(cm)
            return pool

        # ---------------- persistent SBUF + weight prefetch --------------
        consts = _open(name="consts", bufs=1)
        ones64 = consts.tile([128, 64], bf16, tag="ones64", name="ones64")
        strictL = consts.tile([128, 128], bf16, tag="sL", name="sL")
        negI = consts.tile([128, 128], bf16, tag="nI", name="nI")
        b1t = consts.tile([128, FC], f32, tag="b1t", name="b1t")
        g1b = consts.tile([128, D], f32, tag="g1b", name="g1b")
        be1b = consts.tile([128, D], f32, tag="be1b", name="be1b")
        g2b = consts.tile([128, D], f32, tag="g2b", name="g2b")
        be2b = consts.tile([128, D], f32, tag="be2b", name="be2b")
        epsb = consts.tile([128, 1], f32, tag="eps", name="eps")

        pw = _open(name="pw", bufs=1)
        xTt = [pw.tile([128, DC, 512], bf16, tag=f"xT{c}", name=f"xT{c}")
               for c in range(NBP)]
        wq_sb = pw.tile([128, DC, D], bf16, tag="wq", name="wq")
        wk_sb = pw.tile([128, DC, D], bf16, tag="wk", name="wk")
        wv_sb = pw.tile([128, DC, D], bf16, tag="wv", name="wv")
        wo_sb = pw.tile([128, DC, D], bf16, tag="wo", name="wo")
        w1_sb = pw.tile([128, DC, DFF], bf16, tag="w1", name="w1")
        w2_sb = pw.tile([128, FC, D], bf16, tag="w2", name="w2")
        ln1_sb = pw.tile([128, NT, D], bf16, tag="ln1", name="ln1")

        # latency-critical first on the sync queue
        nc.sync.dma_start(wq_sb[:], wq_d[:])
        nc.sync.dma_start(xTt[0][:], xT_d[0])
        nc.sync.dma_start(wk_sb[:], wk_d[:])
        nc.sync.dma_start(wv_sb[:], wv_d[:])
        for c in range(1, NBP):
            nc.sync.dma_start(xTt[c][:], xT_d[c])
        nc.sync.dma_start(wo_sb[:], wo_d[:])
        # small consts first on the scalar queue (strictL gates the first
        # scores mask matmul), then the big FFN weights in the background
        for t_, d_ in ((strictL, sL_d), (negI, nI_d), (ones64, ones_d),
                       (b1t, b1_d), (g1b, g1_d), (be1b, be1_d),
                       (g2b, g2_d), (be2b, be2_d), (epsb, eps_d)):
            nc.scalar.dma_start(t_[:], d_[:])
        nc.scalar.dma_start(w1_sb[:], w1_d[:])
        nc.scalar.dma_start(w2_sb[:], w2_d[:])

        # ------------------------------ pools -----------------------------
        pqk = _open(name="pqk", bufs=2)
        pvb = _open(name="pvb", bufs=2)
        pPT = _open(name="pPT", bufs=4)
        phT = _open(name="phT", bufs=2)
        pln1T = _open(name="pln1T", bufs=2)
        ph1 = _open(name="ph1", bufs=1)
        prec = _open(name="prec", bufs=3)
        pxs = _open(name="pxs", bufs=5)
        pres = _open(name="pres", bufs=4)
        lntmp = _open(name="lntmp", bufs=3)
        pyout = _open(name="pyout", bufs=3)
        lnstat = _open(name="lnstat", bufs=6)
        pscr = _open(name="pscr", bufs=2, space="PSUM")
        pavd = _open(name="pavd", bufs=2, space="PSUM")
        pbig = _open(name="pbig", bufs=2, space="PSUM")
        pff1 = _open(name="pff1", bufs=2, space="PSUM")

        def ln_core(in_ap, out_dt):
            """Normalize (x-mean)*rstd -> fresh tile (no gamma/beta)."""
            st = lnstat.tile([128, 6], f32, tag="st", name="st")
            nc.vector.bn_stats(st[:], in_ap)
            mv = lnstat.tile([128, 2], f32, tag="mv", name="mv")
            nc.vector.bn_aggr(mv[:], st[:])
            std = lnstat.tile([128, 1], f32, tag="std", name="std")
            nc.scalar.activation(std[:], mv[:, 1:2], AF.Sqrt, bias=epsb[:, 0:1])
            rstd = lnstat.tile([128, 1], f32, tag="rstd", name="rstd")
            nc.vector.reciprocal_approx_fast(rstd[:], std[:])
            nmr = lnstat.tile([128, 1], f32, tag="nmr", name="nmr")
            nc.vector.tensor_scalar_mul(nmr[:], mv[:, 0:1], -1.0)
            tmp = lntmp.tile([128, D], out_dt, tag="lnt", name="lnt")
            nc.vector.tensor_scalar(tmp[:], in_ap, nmr[:, 0:1], rstd[:, 0:1],
                                    ALU.add, ALU.mult)
            return tmp

        ln1T_tiles = [None] * NBP

        def qkv_qk(bp, m, qT2, kT2):
            for w_sb, dst in ((wq_sb, qT2), (wk_sb, kT2)):
                ps = pbig.tile([128, 512], f32, tag="pbig", name="pbig")
                for k in range(DC):
                    mm(ps[:], w_sb[:, k, m * 128:(m + 1) * 128],
                       xTt[bp][:, k, :], start=k == 0, stop=k == DC - 1)
                nc.scalar.copy(dst[:, m, :], ps[:])

        def qkv_v(bp, j, vb4):
            ps = pbig.tile([128, 512], f32, tag="pbig", name="pbig")
            for k in range(DC):
                mm(ps[:], xTt[bp][:, k, j * 128:(j + 1) * 128],
                   wv_sb[:, k, :], start=k == 0, stop=k == DC - 1)
            nc.vector.tensor_scalar_mul(vb4[:, j, :], ps[:], 1.0)

        def emit_ffn(bq):
            """FFN + LN2 + store for batch-pair bq (ln1T/ln1_sb ready).
            FFN1 runs in two 256-token column halves so the tail FFN can
            start as soon as the first two ln1T transposes land."""
            lta, ltb = ln1T_tiles[bq]
            h1 = ph1.tile([128, FC, 512], bf16, tag="h1", name="h1")
            for half, lt in ((0, lta), (1, ltb)):
                cs = slice(half * 256, (half + 1) * 256)
                for m in range(FC):
                    ps = pff1.tile([128, 512], f32, tag="pff1", name="pff1")
                    for k in range(DC):
                        mm(ps[:, 0:256], w1_sb[:, k, m * 128:(m + 1) * 128],
                           lt[:, k, :], start=k == 0, stop=k == DC - 1)
                    if m % 2 == 0:
                        nc.scalar.activation(h1[:, m, cs], ps[:, 0:256],
                                             AF.Relu, bias=b1t[:, m:m + 1])
                    else:
                        nc.vector.tensor_scalar(h1[:, m, cs], ps[:, 0:256],
                                                b1t[:, m:m + 1], 0.0,
                                                ALU.add, ALU.max)
            for j in range(4):
                t = 4 * bq + j
                ps2 = pff1.tile([128, 512], f32, tag="pff1", name="pff1")
                for k in range(FC):
                    mm(ps2[:], h1[:, k, j * 128:(j + 1) * 128], w2_sb[:, k, :],
                       start=k == 0, stop=k == FC - 1)
                res2 = pres.tile([128, D], f32, tag="res", name="res")
                nc.vector.tensor_add(res2[:], ps2[:], ln1_sb[:, t, :])
                tmp2 = ln_core(res2[:], f32)
                yt = pyout.tile([128, D], f32, tag="yt", name="yt")
                nc.gpsimd.tensor_mul(yt[:], tmp2[:], g2b[:])
                nc.gpsimd.tensor_add(yt[:], yt[:], be2b[:])
                nc.sync.dma_start(y_d[t], yt[:])

        def emit_scores(u, qT2, kT2):
            """scores (+PE causal mask) + exp for unit u; returns P tile."""
            bi, pair = u // 4, u % 4
            q0 = bi * 256
            ptr = pPT.tile([128, 2, 2, 256], bf16, tag="ptr", name="ptr")
            nc.gpsimd.memset(ptr[:, :, 1, 0:128], 0.0)
            for hh in range(2):
                r0 = hh * 64
                sc = pscr.tile([128, 512], f32, tag="sc", name="sc")
                # kc0: keys 0:128, all 256 queries; mask q<128 triangle
                mm(sc[:, 0:256], kT2[r0:r0 + 64, pair, q0:q0 + 128],
                   qT2[r0:r0 + 64, pair, q0:q0 + 256],
                   start=True, stop=False)
                mm(sc[:, 0:128], strictL[:], negI[:], start=False, stop=True)
                # kc1: keys 128:256, queries 128:256 only (causal trim)
                mm(sc[:, 384:512], kT2[r0:r0 + 64, pair, q0 + 128:q0 + 256],
                   qT2[r0:r0 + 64, pair, q0 + 128:q0 + 256],
                   start=True, stop=False)
                mm(sc[:, 384:512], strictL[:], negI[:], start=False, stop=True)
                # exp(s/8) straight to the bf16 P tile (masked lanes -> 0)
                nc.scalar.activation(ptr[:, hh, 0, :], sc[:, 0:256],
                                     AF.Exp, scale=0.125)
                nc.scalar.activation(ptr[:, hh, 1, 128:256], sc[:, 384:512],
                                     AF.Exp, scale=0.125)
            return ptr

        def emit_av(u, ptr, vb4, headsT):
            """AV + denominator (hh-packed) + normalize for unit u."""
            bi, pair = u // 4, u % 4
            avd = pavd.tile([128, 512], f32, tag="avd", name="avd")
            for hh in range(2):
                h = 2 * pair + hh
                c0 = hh * 64
                he = slice(h * E, (h + 1) * E)
                o = avd[c0:c0 + 64, :]
                mm(o[:, 0:128], vb4[:, 2 * bi, he], ptr[:, hh, 0, 0:128],
                   start=True, stop=True)
                mm(o[:, 128:256], vb4[:, 2 * bi, he], ptr[:, hh, 0, 128:256],
                   start=True, stop=False)
                mm(o[:, 128:256], vb4[:, 2 * bi + 1, he],
                   ptr[:, hh, 1, 128:256], start=False, stop=True)
                mm(o[:, 256:384], ones64[:], ptr[:, hh, 0, 0:128],
                   start=True, stop=True)
                mm(o[:, 384:512], ones64[:], ptr[:, hh, 0, 128:256],
                   start=True, stop=False)
                mm(o[:, 384:512], ones64[:], ptr[:, hh, 1, 128:256],
                   start=False, stop=True)
            rec = prec.tile([128, 256], f32, tag="rec", name="rec")
            nc.vector.reciprocal_approx_fast(rec[:], avd[:, 256:512])
            nc.vector.tensor_mul(headsT[:, pair, bi * 256:(bi + 1) * 256],
                                 avd[:, 0:256], rec[:])

        def proj_mm(j, headsT):
            pool = pbig if j < 2 else pscr
            tag = "pbig" if j < 2 else "sc"
            ps = pool.tile([128, 512], f32, tag=tag, name=tag)
            for k in range(DC):
                mm(ps[:], headsT[:, k, j * 128:(j + 1) * 128],
                   wo_sb[:, k, :], start=k == 0, stop=k == DC - 1)
            return ps

        def proj_ln(bp, j, ps, ln1T_ab, xin):
            t = 4 * bp + j
            res = pres.tile([128, D], f32, tag="res", name="res")
            nc.vector.tensor_add(res[:], ps[:], xin[:])
            tmp = ln_core(res[:], bf16)
            lt = ln1T_ab[j // 2]
            jj = j % 2
            (nc.sync if j % 2 else nc.scalar).dma_start_transpose(
                lt[:, :, jj * 128:(jj + 1) * 128], tmp[:])
            nc.gpsimd.tensor_mul(ln1_sb[:, t, :], tmp[:], g1b[:])
            nc.gpsimd.tensor_add(ln1_sb[:, t, :], ln1_sb[:, t, :], be1b[:])

        # ------------------------------ main loop -------------------------
        for bp in range(NBP):
            xins = []
            for j in range(4):
                xin = pxs.tile([128, D], f32, tag="xs", name="xs")
                nc.sync.dma_start(xin[:], x_d[4 * bp + j])
                xins.append(xin)
            qT2 = pqk.tile([128, DC, 512], bf16, tag="q", name="qT2")
            kT2 = pqk.tile([128, DC, 512], bf16, tag="k", name="kT2")
            vb4 = pvb.tile([128, 4, D], bf16, tag="v", name="vb4")
            headsT = phT.tile([128, DC, 512], bf16, tag="hT", name="hT")
            ln1T_ab = (
                pln1T.tile([128, DC, 256], bf16, tag="l1Ta", name="l1Ta"),
                pln1T.tile([128, DC, 256], bf16, tag="l1Tb", name="l1Tb"))
            ln1T_tiles[bp] = ln1T_ab

            qkv_qk(bp, 0, qT2, kT2)
            qkv_qk(bp, 1, qT2, kT2)
            qkv_v(bp, 0, vb4)
            qkv_v(bp, 1, vb4)
            if bp > 0:
                emit_ffn(bp - 1)
            ptrs = [None] * 8
            ptrs[0] = emit_scores(0, qT2, kT2)
            qkv_qk(bp, 2, qT2, kT2)
            ptrs[1] = emit_scores(1, qT2, kT2)
            emit_av(0, ptrs[0], vb4, headsT)
            qkv_qk(bp, 3, qT2, kT2)
            ptrs[2] = emit_scores(2, qT2, kT2)
            emit_av(1, ptrs[1], vb4, headsT)
            qkv_v(bp, 2, vb4)
            qkv_v(bp, 3, vb4)
            ptrs[3] = emit_scores(3, qT2, kT2)
            emit_av(2, ptrs[2], vb4, headsT)
            ptrs[4] = emit_scores(4, qT2, kT2)
            emit_av(3, ptrs[3], vb4, headsT)
            ptrs[5] = emit_scores(5, qT2, kT2)
            pjs = [None] * 4
            pjs[0] = proj_mm(0, headsT)
            emit_av(4, ptrs[4], vb4, headsT)
            ptrs[6] = emit_scores(6, qT2, kT2)
            pjs[1] = proj_mm(1, headsT)
            emit_av(5, ptrs[5], vb4, headsT)
            ptrs[7] = emit_scores(7, qT2, kT2)
            emit_av(6, ptrs[6], vb4, headsT)
            emit_av(7, ptrs[7], vb4, headsT)
            pjs[2] = proj_mm(2, headsT)
            pjs[3] = proj_mm(3, headsT)
            for j in range(4):
                proj_ln(bp, j, pjs[j], ln1T_ab, xins[j])
        emit_ffn(NBP - 1)

        for cm in reversed(_cms):
            cm.__exit__(None, None, None)

    nc.finalize()
    return nc


def _host_prep(inputs):
    """Build the per-core in_maps from full inputs."""
    import ml_dtypes
    bf = ml_dtypes.bfloat16
    x = np.ascontiguousarray(np.asarray(inputs["x"], np.float32))
    Wq = np.asarray(inputs["Wq"], np.float32)
    Wk = np.asarray(inputs["Wk"], np.float32)
    Wv = np.asarray(inputs["Wv"], np.float32)
    Wo = np.asarray(inputs["Wo"], np.float32)
    W1 = np.asarray(inputs["W1"], np.float32)
    b1 = np.asarray(inputs["b1"], np.float32)
    W2 = np.asarray(inputs["W2"], np.float32)
    b2 = np.asarray(inputs["b2"], np.float32)
    g1 = np.asarray(inputs["ln1_g"], np.float32)
    be1 = np.asarray(inputs["ln1_b"], np.float32)
    g2 = np.asarray(inputs["ln2_g"], np.float32)
    be2 = np.asarray(inputs["ln2_b"], np.float32)

    def chunk_k(w, dt):   # [K, M] -> [128, K//128, M]
        K, M = w.shape
        return np.ascontiguousarray(
            w.reshape(K // 128, 128, M).transpose(1, 0, 2).astype(dt))

    W1g = g1[:, None] * W1                 # fold ln1 gamma into W1
    b1_eff = b1 + be1 @ W1                 # fold ln1 beta into FFN1 bias

    common = {
        "wq": chunk_k(Wq.transpose(1, 0, 2).reshape(D, H * E), bf),
        "wk": chunk_k(Wk.transpose(1, 0, 2).reshape(D, H * E), bf),
        "wv": chunk_k(Wv.transpose(1, 0, 2).reshape(D, H * E), bf),
        "wo": chunk_k(Wo, bf),
        "w1": chunk_k(W1g, bf),
        "w2": chunk_k(W2, bf),
        "b1t": np.ascontiguousarray(b1_eff.reshape(FC, 128).T
                                    .astype(np.float32)),
        "g1b": np.ascontiguousarray(np.tile(g1, (128, 1))),
        "be1b": np.ascontiguousarray(np.tile(be1 + b2, (128, 1))),
        "g2b": np.ascontiguousarray(np.tile(g2, (128, 1))),
        "be2b": np.ascontiguousarray(np.tile(be2, (128, 1))),
        "strictL": (np.arange(128)[None, :] > np.arange(128)[:, None])
            .astype(bf),
        "negI": (NEG * np.eye(128)).astype(bf),
        "ones64": np.ones((128, 64), bf),
        "eps": np.full((128, 1), 1e-5, np.float32),
    }
    in_maps = []
    for core in range(NCORES):
        xc = x[core * BPC:(core + 1) * BPC].reshape(NT, 128, D)
        xTc = np.ascontiguousarray(
            xc.reshape(TOK, D).T.reshape(DC, 128, NBP, 512)
            .transpose(2, 1, 0, 3).astype(bf))
        in_maps.append({"x": np.ascontiguousarray(xc), "xT": xTc, **common})
    return in_maps


def _get_program():
    global _cached
    if _cached is None:
        _cached = _build_program()
    return _cached


def _run(inputs, trace=False):
    from concourse.bass_utils import run_bass_kernel_spmd
    nc = _get_program()
    in_maps = _host_prep(inputs)
    res = run_bass_kernel_spmd(nc, in_maps, list(range(NCORES)), trace=trace)
    outs = [res.results[i]["y"].reshape(BPC, T, D) for i in range(NCORES)]
    return np.concatenate(outs, 0).astype(np.float32), res


def kernel(**inputs):
    out, _ = _run(inputs, trace=False)
    return out
